# revision 59
# baseline (speedup 1.0000x reference)
"""LocalGraphAttention TRN2 kernel: 8-core SPMD (data-parallel B x head-parallel).

Per core c: b = c//2, heads = 4*(c%2) + [0..3]. Keys on partitions so the
softmax key-reduction and PV ride the PE:
  xT (D,G) -> QT (A-prescaled)/KT (128=4h*32, G); S^T = K @ Q^T per head
  (row-banded tile_position, contraction 32) into PSUM as u = A*s with
  A = 128*log2(e)*scale.

The exp+mask elementwise work (16.8M elems/core) is split across engines by a
route pattern (per (qg,kb,pair) tile), all consistently scaled by 2^40:
  'M': PE accumulates +5120*mask01 into the score PSUM (identity matmul),
       then ScalarE exp (masked entries come out 2^40 smaller -> negligible).
  'A': ScalarE exp with bias 40*ln2, VectorE multiply by mask01.
  'G': same but mask multiply on GpSimd.
  'S': single VectorE scalar_tensor_tensor: int16((u + 21369)*mask01),
       bitcast to bf16 = Schraudolph exp2 with mask folded (sawtooth ~4%,
       cancels partially in softmax; fraction kept ~3/8).
PV matmul per head uses V padded with a ones column -> rowsum rides free in
the same accumulation. Normalize via ind-broadcast matmul + DVE
reciprocal_approx_fast. V-bias and out-bias folded into host-side b_out.
"""
import sys
import numpy as np
import ml_dtypes

sys.path.insert(0, "/opt/trn_rl_repo")

from contextlib import ExitStack

import concourse.bass as bass
import concourse.mybir as mybir
import concourse.tile as tile
from concourse import bacc
from concourse.bass_utils import run_bass_kernel_spmd

BF16 = ml_dtypes.bfloat16
G = 2048
D = 256
NH = 8
DH = 32
B = 4
NCORES = 8
SCALE = 1.0 / np.sqrt(np.float32(DH))
KB = G // 128   # 16 key blocks
QG = G // 512   # 4 query groups

A_FOLD = 128.0 * np.log2(np.e) * SCALE          # folded into Wq/bq on host
ACT_SCALE = float(np.log(2.0) / 128.0)           # exp(u*ACT_SCALE) = 2^(u/128)
ACT_BIAS = float(40.0 * np.log(2.0))             # *2^40 for A/G routes
MASK_GAIN = 5120.0                               # 40*128, for 'M' route
B_SCHR = 16249.0 + 5120.0                        # Schraudolph bias (incl 2^40)

# route per (qg,kb,pair) tile; for fixed (qg,pair) the kb loop sweeps the
# full multiset, so every head/query row sees the same route mix.
ROUTE = "SASGSASMSASGSASM"   # 8 S, 4 A, 2 G, 2 M  (len 16)


def build_nc():
    nc = bacc.Bacc("TRN2", target_bir_lowering=False, debug=False)
    dt = mybir.dt
    xT = nc.declare_dram_parameter("xT", [D, G], dt.bfloat16, isOutput=False)
    Wq = nc.declare_dram_parameter("Wq", [D, 128], dt.bfloat16, isOutput=False)
    Wk = nc.declare_dram_parameter("Wk", [D, 128], dt.bfloat16, isOutput=False)
    Wv = nc.declare_dram_parameter("Wv", [D, 128], dt.bfloat16, isOutput=False)
    bq = nc.declare_dram_parameter("bq", [128, 1], dt.float32, isOutput=False)
    bk = nc.declare_dram_parameter("bk", [128, 1], dt.float32, isOutput=False)
    M01T = nc.declare_dram_parameter("M01T", [G, G], dt.bfloat16, isOutput=False)
    EYE = nc.declare_dram_parameter("EYE", [128, 128], dt.bfloat16, isOutput=False)
    Wo = nc.declare_dram_parameter("Wo", [128, D], dt.bfloat16, isOutput=False)
    OUT = nc.declare_dram_parameter("out", [D, G], dt.float32, isOutput=True)

    with tile.TileContext(nc) as tc, ExitStack() as ctx:
        singles = ctx.enter_context(tc.tile_pool(name="singles", bufs=1))
        maskp = ctx.enter_context(tc.tile_pool(name="maskp", bufs=KB))
        vp = ctx.enter_context(tc.tile_pool(name="vp", bufs=KB))
        work = ctx.enter_context(tc.tile_pool(name="work", bufs=12))
        rsp = ctx.enter_context(tc.tile_pool(name="rsp", bufs=4))
        rnorm = ctx.enter_context(tc.tile_pool(name="rnorm", bufs=2))
        psq = ctx.enter_context(tc.tile_pool(name="psq", bufs=3, space="PSUM"))
        ppv = ctx.enter_context(tc.tile_pool(name="ppv", bufs=1, space="PSUM"))

        # ---- resident loads ----
        xt = []
        for kc in range(2):
            t = singles.tile([128, G], dt.bfloat16, tag=f"xt{kc}")
            for ch in range(4):
                csl = slice(512 * ch, 512 * (ch + 1))
                nc.sync.dma_start(out=t[:, csl],
                                  in_=xT[128 * kc:128 * (kc + 1), csl])
            xt.append(t)
        m_sb = []
        for kb in range(KB):
            t = maskp.tile([128, G], dt.bfloat16, tag="mask",
                           name=f"m_{kb}")
            m_sb.append(t)
        # mask0's qg0 columns dispatched early (small, gates the 1st unit)
        nc.sync.dma_start(out=m_sb[0][:, 0:512], in_=M01T[0:128, 0:512])
        wght = {}
        for name, p in (("wq", Wq), ("wk", Wk), ("wv", Wv)):
            t = singles.tile([128, 256], dt.bfloat16, tag=f"{name}t")
            nc.sync.dma_start(
                out=t[:].rearrange("p (a q) -> p a q", a=2),
                in_=p[:].rearrange("(a p) q -> p a q", p=128))
            wght[f"{name}0"] = t[:, 0:128]
            wght[f"{name}1"] = t[:, 128:256]
        bq_sb = singles.tile([128, 1], dt.float32, tag="bq")
        nc.sync.dma_start(out=bq_sb[:], in_=bq[:])
        bk_sb = singles.tile([128, 1], dt.float32, tag="bk")
        nc.sync.dma_start(out=bk_sb[:], in_=bk[:])
        nc.sync.dma_start(out=m_sb[0][:, 512:2048], in_=M01T[0:128, 512:2048])
        nc.sync.dma_start(out=m_sb[1][:], in_=M01T[128:256, :])
        eye_sb = singles.tile([128, 128], dt.bfloat16, tag="eye")
        nc.sync.dma_start(out=eye_sb[:], in_=EYE[:])
        wo_sb = singles.tile([128, D], dt.bfloat16, tag="wo")
        nc.sync.dma_start(out=wo_sb[:], in_=Wo[:])
        ones32 = singles.tile([128, 32], dt.bfloat16, tag="ones32")
        nc.vector.memset(ones32[:], 1.0)
        for kb in range(2, KB):
            nc.sync.dma_start(out=m_sb[kb][:],
                              in_=M01T[128 * kb:128 * (kb + 1), :])
        bias40 = singles.tile([128, 1], dt.float32, tag="bias40")
        nc.vector.memset(bias40[:], ACT_BIAS)
        bias0 = singles.tile([128, 1], dt.float32, tag="bias0")
        nc.vector.memset(bias0[:], 0.0)

        # ---- QKV projections (slice 0 upfront; rest interleaved into qg0's
        # attention stream so the first attention unit starts early) ----
        qt_sb = singles.tile([128, G], dt.bfloat16, tag="qt")
        kt_sb = singles.tile([128, G], dt.bfloat16, tag="kt")
        v_sb = singles.tile([128, G], dt.bfloat16, tag="vsb")

        def proj_qk(dst, wn, b_sb, sidx):
            ps = psq.tile([128, 1024], dt.float32, tag="sq",
                          name=f"pj_{wn}_{sidx}")
            sl = slice(512 * sidx, 512 * (sidx + 1))
            nc.tensor.matmul(ps[:, 0:512], wght[wn + "0"],
                             xt[0][:, sl], start=True, stop=False)
            nc.tensor.matmul(ps[:, 0:512], wght[wn + "1"],
                             xt[1][:, sl], start=False, stop=True)
            nc.scalar.activation(dst[:, sl], ps[:, 0:512],
                                 mybir.ActivationFunctionType.Identity,
                                 bias=b_sb[:], scale=1.0)

        def proj_v(grp):
            ps = psq.tile([128, 1024], dt.float32, tag="sq",
                          name=f"pjv_{grp}")
            for k4 in range(4):
                kb = 4 * grp + k4
                sl = slice(128 * kb, 128 * (kb + 1))
                osl = slice(128 * k4, 128 * (k4 + 1))
                nc.tensor.matmul(ps[:, osl], xt[0][:, sl], wght["wv0"],
                                 start=True, stop=False)
                nc.tensor.matmul(ps[:, osl], xt[1][:, sl], wght["wv1"],
                                 start=False, stop=True)
            nc.vector.tensor_copy(v_sb[:, 512 * grp:512 * (grp + 1)],
                                  ps[:, 0:512])

        proj_qk(kt_sb, "wk", bk_sb, 0)
        proj_qk(qt_sb, "wq", bq_sb, 0)
        proj_v(0)
        deferred_proj = [
            lambda: proj_qk(kt_sb, "wk", bk_sb, 1),
            lambda: proj_v(1),
            lambda: proj_qk(kt_sb, "wk", bk_sb, 2),
            lambda: proj_v(2),
            lambda: proj_qk(kt_sb, "wk", bk_sb, 3),
            lambda: proj_v(3),
            lambda: proj_qk(qt_sb, "wq", bq_sb, 1),
            lambda: proj_qk(qt_sb, "wq", bq_sb, 2),
            lambda: proj_qk(qt_sb, "wq", bq_sb, 3),
        ]

        # ---- attention ----
        yn = singles.tile([128, G], dt.bfloat16, tag="yn")

        def emit_outproj(qg):
            # out projection for qg (deferred into the next qg's stream so
            # the PE does not stall on the normalize chain)
            oqsl = slice(512 * qg, 512 * (qg + 1))
            for mt in range(2):
                ps = psq.tile([128, 1024], dt.float32, tag="sq",
                              name=f"op_{qg}_{mt}")
                nc.tensor.matmul(ps[:, 0:512],
                                 wo_sb[:, 128 * mt:128 * (mt + 1)],
                                 yn[:, oqsl], start=True, stop=True)
                ot = work.tile([128, 1024], dt.bfloat16, tag="ot",
                               name=f"ot_{qg}_{mt}")
                otf = ot[:, 0:1024].bitcast(dt.float32)
                nc.scalar.copy(otf, ps[:, 0:512])
                nc.sync.dma_start(out=OUT[128 * mt:128 * (mt + 1), oqsl],
                                  in_=otf)

        pending_op = None
        for qg in range(QG):
            qsl = slice(512 * qg, 512 * (qg + 1))
            pv = ppv.tile([128, 512], dt.float32, tag="pv",
                          name=f"pv_{qg}")
            rs = ppv.tile([128, 512], dt.float32, tag="rs",
                          name=f"rs_{qg}")
            def emit_pv_rs(kb, rhss):
                # PV wave: 4 heads, 32-col tiles coalesce on the psum port
                for h in range(4):
                    nc.tensor.matmul(
                        pv[32 * h:32 * (h + 1), :],
                        v_sb[:, 128 * kb + 32 * h:128 * kb + 32 * (h + 1)],
                        rhss[h // 2][h % 2],
                        start=(kb == 0), stop=(kb == KB - 1),
                        tile_position=(0, 32 * h), skip_group_check=True)
                # rowsum wave: ones-block -> rowsum replicated over the band
                for h in range(4):
                    nc.tensor.matmul(
                        rs[32 * h:32 * (h + 1), :],
                        ones32[:], rhss[h // 2][h % 2],
                        start=(kb == 0), stop=(kb == KB - 1),
                        tile_position=(0, 32 * h), skip_group_check=True)

            pending = []
            for kb in range(KB):
                routes = [ROUTE[(kb + 5 * pair + 3 * qg) % len(ROUTE)]
                          for pair in range(2)]
                St = []
                # all 4 score MMs back-to-back (4 PE row bands pipeline,
                # LDWEIGHTS for band h+1 hides under band h's stream)
                for pair in range(2):
                    S = psq.tile([128, 1024], dt.float32, tag="sq",
                                 name=f"S_{qg}_{kb}_{pair}")
                    St.append(S)
                for pair in range(2):
                    for j in range(2):
                        h = 2 * pair + j
                        hsl = slice(32 * h, 32 * (h + 1))
                        nc.tensor.matmul(
                            St[pair][:, 512 * j:512 * (j + 1)],
                            kt_sb[hsl, 128 * kb:128 * (kb + 1)],
                            qt_sb[hsl, qsl],
                            start=True, stop=(routes[pair] != "M"),
                            tile_position=(32 * h, 0))
                for pair in range(2):
                    if routes[pair] == "M":
                        for j in range(2):
                            nc.tensor.matmul(
                                St[pair][:, 512 * j:512 * (j + 1)], eye_sb[:],
                                m_sb[kb][:, qsl], start=False, stop=True,
                                skip_group_check=True)
                mb = m_sb[kb][:, qsl].rearrange(
                    "p (a b) -> p a b", a=1).broadcast_to((128, 2, 512))
                rhss = []
                for pair in range(2):
                    route = routes[pair]
                    S = St[pair]
                    if route in "MAG":
                        e = work.tile([128, 1024], dt.bfloat16, tag="e",
                                      name=f"e_{qg}_{kb}_{pair}")
                        nc.scalar.activation(
                            e[:], S[:], mybir.ActivationFunctionType.Exp,
                            scale=ACT_SCALE,
                            bias=bias0[:] if route == "M" else bias40[:])
                        if route == "M":
                            em_t = e
                        else:
                            em_t = work.tile([128, 1024], dt.bfloat16,
                                             tag="em",
                                             name=f"em_{qg}_{kb}_{pair}")
                            e3 = e[:].rearrange("p (a b) -> p a b", a=2)
                            o3 = em_t[:].rearrange("p (a b) -> p a b", a=2)
                            eng = nc.vector if route == "A" else nc.gpsimd
                            eng.tensor_mul(o3, e3, mb)
                        rhss.append((em_t[:, 0:512], em_t[:, 512:1024]))
                    else:  # 'S': fused Schraudolph + mask on VectorE
                        emi = work.tile([128, 1024], dt.int16, tag="emi",
                                        name=f"emi_{qg}_{kb}_{pair}")
                        s3 = S[:].rearrange("p (a b) -> p a b", a=2)
                        o3 = emi[:].rearrange("p (a b) -> p a b", a=2)
                        nc.vector.scalar_tensor_tensor(
                            o3, s3, B_SCHR, mb,
                            op0=mybir.AluOpType.add,
                            op1=mybir.AluOpType.mult)
                        rhss.append((emi[:, 0:512].bitcast(dt.bfloat16),
                                     emi[:, 512:1024].bitcast(dt.bfloat16)))
                # software pipelining: PV/RS run two kb iterations behind the
                # scores+elementwise, so the PE has score work while the
                # elementwise engines (incl. slow GpSimd) produce em(kb)
                pending.append((kb, rhss))
                if len(pending) > 2:
                    emit_pv_rs(*pending.pop(0))
                if qg == 0 and 1 <= kb <= len(deferred_proj):
                    deferred_proj[kb - 1]()
                if kb == 2 and pending_op is not None:
                    emit_outproj(pending_op)
                    pending_op = None
            for item in pending:
                emit_pv_rs(*item)
            pending = []
            # normalize: rs holds per-head rowsums replicated over each
            # 32-row band, so it is already the broadcast denominator
            rinv = rnorm.tile([128, 512], dt.float32, tag="rinv",
                              name=f"rinv_{qg}")
            nc.vector.reciprocal_approx_fast(rinv[:], rs[:])
            nc.vector.tensor_mul(yn[:, qsl], pv[:], rinv[:])
            pending_op = qg
        emit_outproj(QG - 1)
    nc.finalize()
    return nc


_NC_CACHE = None


def kernel(x, allow_mask_bool, W_qkv, b_qkv, W_out, b_out):
    global _NC_CACHE
    x = np.asarray(x, np.float32)
    allow = np.asarray(allow_mask_bool)
    W_qkv = np.asarray(W_qkv, np.float32)
    b_qkv = np.asarray(b_qkv, np.float32)
    W_out = np.asarray(W_out, np.float32)
    b_out = np.asarray(b_out, np.float32)

    M01T = np.ascontiguousarray(allow.T).astype(BF16)
    EYEm = (MASK_GAIN * np.eye(128, dtype=np.float32)).astype(BF16)
    in_maps = []
    for c in range(NCORES):
        b = c // 2
        hs = [4 * (c % 2) + i for i in range(4)]
        qcols = np.concatenate([np.arange(32 * h, 32 * h + 32) for h in hs])
        m = {
            "xT": np.ascontiguousarray(x[b].T).astype(BF16),
            "Wq": np.ascontiguousarray(W_qkv[:, qcols] * A_FOLD).astype(BF16),
            "Wk": np.ascontiguousarray(W_qkv[:, 256 + qcols]).astype(BF16),
            "Wv": np.ascontiguousarray(W_qkv[:, 512 + qcols]).astype(BF16),
            "bq": np.ascontiguousarray(
                (b_qkv[qcols] * A_FOLD)[:, None]).astype(np.float32),
            "bk": np.ascontiguousarray(
                b_qkv[256 + qcols][:, None]).astype(np.float32),
            "M01T": M01T,
            "EYE": EYEm,
            "Wo": np.ascontiguousarray(W_out[qcols, :]).astype(BF16),
        }
        in_maps.append(m)

    global LAST_IN_MAPS
    LAST_IN_MAPS = in_maps
    if _NC_CACHE is None:
        _NC_CACHE = build_nc()
    res = run_bass_kernel_spmd(_NC_CACHE, in_maps, core_ids=list(range(NCORES)))
    out = np.zeros((B, G, D), np.float32)
    for c in range(NCORES):
        out[c // 2] += res.results[c]["out"].T
    # v-bias and out-bias folded here: y = softmax(S) @ (xWv) + bv exactly,
    # so out += W_out^T bv + b_out
    out += (b_qkv[512:768] @ W_out + b_out)[None, None, :]
    return out


if __name__ == "__main__":
    rng = np.random.default_rng(0)
    ins = {
        "x": rng.standard_normal((B, G, D), dtype=np.float32),
        "allow_mask_bool": rng.random((G, G)) < 0.5,
        "W_qkv": rng.standard_normal((D, 3 * D), dtype=np.float32) * 0.06,
        "b_qkv": rng.standard_normal(3 * D).astype(np.float32) * 0.06,
        "W_out": rng.standard_normal((D, D), dtype=np.float32) * 0.06,
        "b_out": rng.standard_normal(D).astype(np.float32) * 0.06,
    }
    ins["allow_mask_bool"] |= np.eye(G, dtype=bool)
    out = kernel(**ins)
    print("kernel ran, out shape", out.shape)


# revision 60
# speedup vs baseline: 1.0374x; 1.0374x over previous
"""LocalGraphAttention TRN2 kernel: 8-core SPMD (data-parallel B x head-parallel).

Per core c: b = c//2, heads = 4*(c%2) + [0..3]. Keys on partitions so the
softmax key-reduction and PV ride the PE:
  xT (D,G) -> QT (A-prescaled)/KT (128=4h*32, G); S^T = K @ Q^T per head
  (row-banded tile_position, contraction 32) into PSUM as u = A*s with
  A = 128*log2(e)*scale.

The exp+mask elementwise work (16.8M elems/core) is split across engines by a
route pattern (per (qg,kb,pair) tile), all consistently scaled by 2^40:
  'M': PE accumulates +5120*mask01 into the score PSUM (identity matmul),
       then ScalarE exp (masked entries come out 2^40 smaller -> negligible).
  'A': ScalarE exp with bias 40*ln2, VectorE multiply by mask01.
  'G': same but mask multiply on GpSimd.
  'S': single VectorE scalar_tensor_tensor: int16((u + 21369)*mask01),
       bitcast to bf16 = Schraudolph exp2 with mask folded (sawtooth ~4%,
       cancels partially in softmax; fraction kept ~3/8).
PV matmul per head uses V padded with a ones column -> rowsum rides free in
the same accumulation. Normalize via ind-broadcast matmul + DVE
reciprocal_approx_fast. V-bias and out-bias folded into host-side b_out.
"""
import sys
import numpy as np
import ml_dtypes

sys.path.insert(0, "/opt/trn_rl_repo")

from contextlib import ExitStack

import concourse.bass as bass
import concourse.mybir as mybir
import concourse.tile as tile
from concourse import bacc
from concourse.bass_utils import run_bass_kernel_spmd

BF16 = ml_dtypes.bfloat16
G = 2048
D = 256
NH = 8
DH = 32
B = 4
NCORES = 8
SCALE = 1.0 / np.sqrt(np.float32(DH))
KB = G // 128   # 16 key blocks
QG = G // 512   # 4 query groups

A_FOLD = 128.0 * np.log2(np.e) * SCALE          # folded into Wq/bq on host
ACT_SCALE = float(np.log(2.0) / 128.0)           # exp(u*ACT_SCALE) = 2^(u/128)
ACT_BIAS = float(40.0 * np.log(2.0))             # *2^40 for A/G routes
MASK_GAIN = 5120.0                               # 40*128, for 'M' route
B_SCHR = 16249.0 + 5120.0                        # Schraudolph bias (incl 2^40)

# route per (qg,kb,pair) tile; for fixed (qg,pair) the kb loop sweeps the
# full multiset, so every head/query row sees the same route mix.
ROUTE = "SASGSMSMSASGSMSM"   # 8 S, 2 A, 2 G, 4 M  (len 16)


def build_nc():
    nc = bacc.Bacc("TRN2", target_bir_lowering=False, debug=False)
    dt = mybir.dt
    xT = nc.declare_dram_parameter("xT", [D, G], dt.bfloat16, isOutput=False)
    Wq = nc.declare_dram_parameter("Wq", [D, 128], dt.bfloat16, isOutput=False)
    Wk = nc.declare_dram_parameter("Wk", [D, 128], dt.bfloat16, isOutput=False)
    Wv = nc.declare_dram_parameter("Wv", [D, 128], dt.bfloat16, isOutput=False)
    bq = nc.declare_dram_parameter("bq", [128, 1], dt.float32, isOutput=False)
    bk = nc.declare_dram_parameter("bk", [128, 1], dt.float32, isOutput=False)
    M01T = nc.declare_dram_parameter("M01T", [G, G], dt.bfloat16, isOutput=False)
    EYE = nc.declare_dram_parameter("EYE", [128, 128], dt.bfloat16, isOutput=False)
    Wo = nc.declare_dram_parameter("Wo", [128, D], dt.bfloat16, isOutput=False)
    OUT = nc.declare_dram_parameter("out", [D, G], dt.float32, isOutput=True)

    with tile.TileContext(nc) as tc, ExitStack() as ctx:
        singles = ctx.enter_context(tc.tile_pool(name="singles", bufs=1))
        maskp = ctx.enter_context(tc.tile_pool(name="maskp", bufs=KB))
        vp = ctx.enter_context(tc.tile_pool(name="vp", bufs=KB))
        work = ctx.enter_context(tc.tile_pool(name="work", bufs=12))
        rsp = ctx.enter_context(tc.tile_pool(name="rsp", bufs=4))
        rnorm = ctx.enter_context(tc.tile_pool(name="rnorm", bufs=2))
        psq = ctx.enter_context(tc.tile_pool(name="psq", bufs=3, space="PSUM"))
        ppv = ctx.enter_context(tc.tile_pool(name="ppv", bufs=1, space="PSUM"))

        # ---- resident loads ----
        xt = []
        for kc in range(2):
            t = singles.tile([128, G], dt.bfloat16, tag=f"xt{kc}")
            for ch in range(4):
                csl = slice(512 * ch, 512 * (ch + 1))
                nc.sync.dma_start(out=t[:, csl],
                                  in_=xT[128 * kc:128 * (kc + 1), csl])
            xt.append(t)
        m_sb = []
        for kb in range(KB):
            t = maskp.tile([128, G], dt.bfloat16, tag="mask",
                           name=f"m_{kb}")
            m_sb.append(t)
        # mask0's qg0 columns dispatched early (small, gates the 1st unit)
        nc.sync.dma_start(out=m_sb[0][:, 0:512], in_=M01T[0:128, 0:512])
        wght = {}
        for name, p in (("wq", Wq), ("wk", Wk), ("wv", Wv)):
            t = singles.tile([128, 256], dt.bfloat16, tag=f"{name}t")
            nc.sync.dma_start(
                out=t[:].rearrange("p (a q) -> p a q", a=2),
                in_=p[:].rearrange("(a p) q -> p a q", p=128))
            wght[f"{name}0"] = t[:, 0:128]
            wght[f"{name}1"] = t[:, 128:256]
        bq_sb = singles.tile([128, 1], dt.float32, tag="bq")
        nc.sync.dma_start(out=bq_sb[:], in_=bq[:])
        bk_sb = singles.tile([128, 1], dt.float32, tag="bk")
        nc.sync.dma_start(out=bk_sb[:], in_=bk[:])
        nc.sync.dma_start(out=m_sb[0][:, 512:2048], in_=M01T[0:128, 512:2048])
        nc.sync.dma_start(out=m_sb[1][:], in_=M01T[128:256, :])
        eye_sb = singles.tile([128, 128], dt.bfloat16, tag="eye")
        nc.sync.dma_start(out=eye_sb[:], in_=EYE[:])
        wo_sb = singles.tile([128, D], dt.bfloat16, tag="wo")
        nc.sync.dma_start(out=wo_sb[:], in_=Wo[:])
        ones32 = singles.tile([128, 32], dt.bfloat16, tag="ones32")
        nc.vector.memset(ones32[:], 1.0)
        for kb in range(2, KB):
            nc.sync.dma_start(out=m_sb[kb][:],
                              in_=M01T[128 * kb:128 * (kb + 1), :])
        bias40 = singles.tile([128, 1], dt.float32, tag="bias40")
        nc.vector.memset(bias40[:], ACT_BIAS)
        bias0 = singles.tile([128, 1], dt.float32, tag="bias0")
        nc.vector.memset(bias0[:], 0.0)

        # ---- QKV projections (slice 0 upfront; rest interleaved into qg0's
        # attention stream so the first attention unit starts early) ----
        qt_sb = singles.tile([128, G], dt.bfloat16, tag="qt")
        kt_sb = singles.tile([128, G], dt.bfloat16, tag="kt")
        v_sb = singles.tile([128, G], dt.bfloat16, tag="vsb")

        def proj_qk(dst, wn, b_sb, sidx):
            ps = psq.tile([128, 1024], dt.float32, tag="sq",
                          name=f"pj_{wn}_{sidx}")
            sl = slice(512 * sidx, 512 * (sidx + 1))
            nc.tensor.matmul(ps[:, 0:512], wght[wn + "0"],
                             xt[0][:, sl], start=True, stop=False)
            nc.tensor.matmul(ps[:, 0:512], wght[wn + "1"],
                             xt[1][:, sl], start=False, stop=True)
            nc.scalar.activation(dst[:, sl], ps[:, 0:512],
                                 mybir.ActivationFunctionType.Identity,
                                 bias=b_sb[:], scale=1.0)

        def proj_v(grp):
            ps = psq.tile([128, 1024], dt.float32, tag="sq",
                          name=f"pjv_{grp}")
            for k4 in range(4):
                kb = 4 * grp + k4
                sl = slice(128 * kb, 128 * (kb + 1))
                osl = slice(128 * k4, 128 * (k4 + 1))
                nc.tensor.matmul(ps[:, osl], xt[0][:, sl], wght["wv0"],
                                 start=True, stop=False)
                nc.tensor.matmul(ps[:, osl], xt[1][:, sl], wght["wv1"],
                                 start=False, stop=True)
            nc.vector.tensor_copy(v_sb[:, 512 * grp:512 * (grp + 1)],
                                  ps[:, 0:512])

        proj_qk(kt_sb, "wk", bk_sb, 0)
        proj_qk(qt_sb, "wq", bq_sb, 0)
        proj_v(0)
        deferred_proj = [
            lambda: proj_qk(kt_sb, "wk", bk_sb, 1),
            lambda: proj_v(1),
            lambda: proj_qk(kt_sb, "wk", bk_sb, 2),
            lambda: proj_v(2),
            lambda: proj_qk(kt_sb, "wk", bk_sb, 3),
            lambda: proj_v(3),
            lambda: proj_qk(qt_sb, "wq", bq_sb, 1),
            lambda: proj_qk(qt_sb, "wq", bq_sb, 2),
            lambda: proj_qk(qt_sb, "wq", bq_sb, 3),
        ]

        # ---- attention ----
        yn = singles.tile([128, G], dt.bfloat16, tag="yn")

        def emit_outproj(qg):
            # out projection for qg (deferred into the next qg's stream so
            # the PE does not stall on the normalize chain)
            oqsl = slice(512 * qg, 512 * (qg + 1))
            for mt in range(2):
                ps = psq.tile([128, 1024], dt.float32, tag="sq",
                              name=f"op_{qg}_{mt}")
                nc.tensor.matmul(ps[:, 0:512],
                                 wo_sb[:, 128 * mt:128 * (mt + 1)],
                                 yn[:, oqsl], start=True, stop=True)
                ot = work.tile([128, 1024], dt.bfloat16, tag="ot",
                               name=f"ot_{qg}_{mt}")
                otf = ot[:, 0:1024].bitcast(dt.float32)
                nc.scalar.copy(otf, ps[:, 0:512])
                nc.sync.dma_start(out=OUT[128 * mt:128 * (mt + 1), oqsl],
                                  in_=otf)

        pending_op = None
        for qg in range(QG):
            qsl = slice(512 * qg, 512 * (qg + 1))
            pv = ppv.tile([128, 512], dt.float32, tag="pv",
                          name=f"pv_{qg}")
            rs = ppv.tile([128, 512], dt.float32, tag="rs",
                          name=f"rs_{qg}")
            def emit_pv_rs(kb, rhss):
                # PV wave: 4 heads, 32-col tiles coalesce on the psum port
                for h in range(4):
                    nc.tensor.matmul(
                        pv[32 * h:32 * (h + 1), :],
                        v_sb[:, 128 * kb + 32 * h:128 * kb + 32 * (h + 1)],
                        rhss[h // 2][h % 2],
                        start=(kb == 0), stop=(kb == KB - 1),
                        tile_position=(0, 32 * h), skip_group_check=True)
                # rowsum wave: ones-block -> rowsum replicated over the band
                for h in range(4):
                    nc.tensor.matmul(
                        rs[32 * h:32 * (h + 1), :],
                        ones32[:], rhss[h // 2][h % 2],
                        start=(kb == 0), stop=(kb == KB - 1),
                        tile_position=(0, 32 * h), skip_group_check=True)

            pending = []
            for kb in range(KB):
                routes = [ROUTE[(kb + 5 * pair + 3 * qg) % len(ROUTE)]
                          for pair in range(2)]
                St = []
                # all 4 score MMs back-to-back (4 PE row bands pipeline,
                # LDWEIGHTS for band h+1 hides under band h's stream)
                for pair in range(2):
                    S = psq.tile([128, 1024], dt.float32, tag="sq",
                                 name=f"S_{qg}_{kb}_{pair}")
                    St.append(S)
                for pair in range(2):
                    for j in range(2):
                        h = 2 * pair + j
                        hsl = slice(32 * h, 32 * (h + 1))
                        nc.tensor.matmul(
                            St[pair][:, 512 * j:512 * (j + 1)],
                            kt_sb[hsl, 128 * kb:128 * (kb + 1)],
                            qt_sb[hsl, qsl],
                            start=True, stop=(routes[pair] != "M"),
                            tile_position=(32 * h, 0))
                for pair in range(2):
                    if routes[pair] == "M":
                        for j in range(2):
                            nc.tensor.matmul(
                                St[pair][:, 512 * j:512 * (j + 1)], eye_sb[:],
                                m_sb[kb][:, qsl], start=False, stop=True,
                                skip_group_check=True)
                mb = m_sb[kb][:, qsl].rearrange(
                    "p (a b) -> p a b", a=1).broadcast_to((128, 2, 512))
                rhss = []
                for pair in range(2):
                    route = routes[pair]
                    S = St[pair]
                    if route in "MAG":
                        e = work.tile([128, 1024], dt.bfloat16, tag="e",
                                      name=f"e_{qg}_{kb}_{pair}")
                        nc.scalar.activation(
                            e[:], S[:], mybir.ActivationFunctionType.Exp,
                            scale=ACT_SCALE,
                            bias=bias0[:] if route == "M" else bias40[:])
                        if route == "M":
                            em_t = e
                        else:
                            em_t = work.tile([128, 1024], dt.bfloat16,
                                             tag="em",
                                             name=f"em_{qg}_{kb}_{pair}")
                            e3 = e[:].rearrange("p (a b) -> p a b", a=2)
                            o3 = em_t[:].rearrange("p (a b) -> p a b", a=2)
                            eng = nc.vector if route == "A" else nc.gpsimd
                            eng.tensor_mul(o3, e3, mb)
                        rhss.append((em_t[:, 0:512], em_t[:, 512:1024]))
                    else:  # 'S': fused Schraudolph + mask on VectorE
                        emi = work.tile([128, 1024], dt.int16, tag="emi",
                                        name=f"emi_{qg}_{kb}_{pair}")
                        s3 = S[:].rearrange("p (a b) -> p a b", a=2)
                        o3 = emi[:].rearrange("p (a b) -> p a b", a=2)
                        nc.vector.scalar_tensor_tensor(
                            o3, s3, B_SCHR, mb,
                            op0=mybir.AluOpType.add,
                            op1=mybir.AluOpType.mult)
                        rhss.append((emi[:, 0:512].bitcast(dt.bfloat16),
                                     emi[:, 512:1024].bitcast(dt.bfloat16)))
                # software pipelining: PV/RS run two kb iterations behind the
                # scores+elementwise, so the PE has score work while the
                # elementwise engines (incl. slow GpSimd) produce em(kb)
                pending.append((kb, rhss))
                if len(pending) > 2:
                    emit_pv_rs(*pending.pop(0))
                if qg == 0 and 1 <= kb <= len(deferred_proj):
                    deferred_proj[kb - 1]()
                if kb == 2 and pending_op is not None:
                    emit_outproj(pending_op)
                    pending_op = None
            for item in pending:
                emit_pv_rs(*item)
            pending = []
            # normalize: rs holds per-head rowsums replicated over each
            # 32-row band, so it is already the broadcast denominator
            rinv = rnorm.tile([128, 512], dt.float32, tag="rinv",
                              name=f"rinv_{qg}")
            nc.vector.reciprocal_approx_fast(rinv[:], rs[:])
            nc.vector.tensor_mul(yn[:, qsl], pv[:], rinv[:])
            pending_op = qg
        emit_outproj(QG - 1)
    nc.finalize()
    return nc


_NC_CACHE = None


def kernel(x, allow_mask_bool, W_qkv, b_qkv, W_out, b_out):
    global _NC_CACHE
    x = np.asarray(x, np.float32)
    allow = np.asarray(allow_mask_bool)
    W_qkv = np.asarray(W_qkv, np.float32)
    b_qkv = np.asarray(b_qkv, np.float32)
    W_out = np.asarray(W_out, np.float32)
    b_out = np.asarray(b_out, np.float32)

    M01T = np.ascontiguousarray(allow.T).astype(BF16)
    EYEm = (MASK_GAIN * np.eye(128, dtype=np.float32)).astype(BF16)
    in_maps = []
    for c in range(NCORES):
        b = c // 2
        hs = [4 * (c % 2) + i for i in range(4)]
        qcols = np.concatenate([np.arange(32 * h, 32 * h + 32) for h in hs])
        m = {
            "xT": np.ascontiguousarray(x[b].T).astype(BF16),
            "Wq": np.ascontiguousarray(W_qkv[:, qcols] * A_FOLD).astype(BF16),
            "Wk": np.ascontiguousarray(W_qkv[:, 256 + qcols]).astype(BF16),
            "Wv": np.ascontiguousarray(W_qkv[:, 512 + qcols]).astype(BF16),
            "bq": np.ascontiguousarray(
                (b_qkv[qcols] * A_FOLD)[:, None]).astype(np.float32),
            "bk": np.ascontiguousarray(
                b_qkv[256 + qcols][:, None]).astype(np.float32),
            "M01T": M01T,
            "EYE": EYEm,
            "Wo": np.ascontiguousarray(W_out[qcols, :]).astype(BF16),
        }
        in_maps.append(m)

    global LAST_IN_MAPS
    LAST_IN_MAPS = in_maps
    if _NC_CACHE is None:
        _NC_CACHE = build_nc()
    res = run_bass_kernel_spmd(_NC_CACHE, in_maps, core_ids=list(range(NCORES)))
    out = np.zeros((B, G, D), np.float32)
    for c in range(NCORES):
        out[c // 2] += res.results[c]["out"].T
    # v-bias and out-bias folded here: y = softmax(S) @ (xWv) + bv exactly,
    # so out += W_out^T bv + b_out
    out += (b_qkv[512:768] @ W_out + b_out)[None, None, :]
    return out


if __name__ == "__main__":
    rng = np.random.default_rng(0)
    ins = {
        "x": rng.standard_normal((B, G, D), dtype=np.float32),
        "allow_mask_bool": rng.random((G, G)) < 0.5,
        "W_qkv": rng.standard_normal((D, 3 * D), dtype=np.float32) * 0.06,
        "b_qkv": rng.standard_normal(3 * D).astype(np.float32) * 0.06,
        "W_out": rng.standard_normal((D, D), dtype=np.float32) * 0.06,
        "b_out": rng.standard_normal(D).astype(np.float32) * 0.06,
    }
    ins["allow_mask_bool"] |= np.eye(G, dtype=bool)
    out = kernel(**ins)
    print("kernel ran, out shape", out.shape)
